# revision 2
# baseline (speedup 1.0000x reference)
"""CQAttention Trainium2 kernel: out = concat([C, A, C*A, C*Bv], -1).

Math (exact, given all-ones masks):
  - sub0 (per-row) and bias are constant along the softmax axis m -> cancel.
  - sub1[m] = sum_d Q[m,d] w4Q[d] folds into the score matmul exactly:
      sim[n,m] = sum_d (C[n,d]*w4mlu[d] + w4Q[d]) * Q[m,d] = sub2 + sub1
  - S1 == S2 == diag(r) E with E = exp(sim), r = 1/rowsum(E).
  - Reassociation halves the Bv cost:
      Bv = S1 S2^T C = diag(r) E (E^T (diag(r) C))
    i.e.  G = diag(r) C;  H = E^T G  (m-part);  Bv = diag(r) (E H).
  - A = diag(r) (E Q).

Implementation:
  - All operands bf16 (matmul rate equals f32r, transposes are cheaper,
    SBUF/DMA halved). exp is computed from an fp32 PSUM sim, so E is
    accurate to bf16 rounding. Tolerance is 2e-2; this lands ~1e-3.
  - exp + rowsum fused in one scalar-engine activation via accum_out.
  - E is produced n-part (lhsT for H); E^T (lhsT for A/Bv) via 32 PE
    transposes per batch, drained 4-at-a-time from one PSUM tile.
  - Host side: inputs cast to bf16, outputs (A, C*A, C*Bv) returned as
    bf16 and upcast on host; the C passthrough block is assembled on the
    host from the original f32 input (pure data movement, no compute).

Sharding: data-parallel over batch; core i handles batches [2i, 2i+1].
"""

import sys

if "/opt/trn_rl_repo" not in sys.path:
    sys.path.insert(0, "/opt/trn_rl_repo")

import numpy as np

B, N, M, D = 16, 1024, 512, 512
NCORES = 8
BPC = B // NCORES  # batches per core
P = 128
NC = N // P  # 8 n-chunks
MC = M // P  # 4 m-chunks
DC = D // P  # 4 d-chunks

_cache = {}


def _build():
    import concourse.bass as bass
    import concourse.tile as tile
    from concourse import bacc, mybir
    from concourse.masks import make_identity

    f32 = mybir.dt.float32
    bf16 = mybir.dt.bfloat16
    ACT = mybir.ActivationFunctionType
    ALU = mybir.AluOpType

    nc = bacc.Bacc("TRN2")
    Cd = nc.dram_tensor("C", [BPC, N, D], bf16, kind="ExternalInput")
    Qd = nc.dram_tensor("Q", [BPC, M, D], bf16, kind="ExternalInput")
    w4Qd = nc.dram_tensor("w4Q", [D, 1], f32, kind="ExternalInput")
    wmlud = nc.dram_tensor("wmlu", [1, 1, D], f32, kind="ExternalInput")
    Ad = nc.dram_tensor("A", [BPC, N, D], bf16, kind="ExternalOutput")
    CAd = nc.dram_tensor("CA", [BPC, N, D], bf16, kind="ExternalOutput")
    CBvd = nc.dram_tensor("CBv", [BPC, N, D], bf16, kind="ExternalOutput")

    with tile.TileContext(nc) as tc:
        with (
            tc.tile_pool(name="consts", bufs=1) as consts,
            tc.tile_pool(name="io", bufs=2) as io,
            tc.tile_pool(name="work", bufs=2) as work,
            tc.tile_pool(name="stage", bufs=3) as stage,
            tc.tile_pool(name="ps_sim", bufs=2, space="PSUM") as ps_sim,
            tc.tile_pool(name="ps_t", bufs=2, space="PSUM") as ps_t,
            tc.tile_pool(name="ps_h", bufs=1, space="PSUM") as ps_h,
        ):
            ident = consts.tile([P, P], f32, tag="ident")
            make_identity(nc, ident)
            ident_b = consts.tile([P, P], bf16, tag="identb")
            nc.vector.tensor_copy(out=ident_b, in_=ident)
            # per-partition weight tables, element [p, e] = w[e*128 + p]
            wmlu_pp = consts.tile([P, DC], f32, tag="wmlu")
            nc.gpsimd.dma_start(
                out=wmlu_pp, in_=bass.AP(tensor=wmlud, offset=0, ap=[[1, P], [P, DC]])
            )
            w4Q_pp = consts.tile([P, DC], f32, tag="w4q")
            nc.gpsimd.dma_start(
                out=w4Q_pp, in_=bass.AP(tensor=w4Qd, offset=0, ap=[[1, P], [P, DC]])
            )

            def alloc(b):
                tl = {"b": b}
                tl["Cb"] = io.tile([P, NC, D], bf16, tag="cb", name="Cb")
                tl["Qb"] = io.tile([P, MC, D], bf16, tag="qb", name="Qb")
                tl["CT"] = work.tile([P, DC, N], bf16, tag="ct", name="CT")
                tl["QT"] = work.tile([P, DC, M], bf16, tag="qt", name="QT")
                tl["E"] = work.tile([P, NC, M], bf16, tag="e", name="E")
                tl["ET"] = work.tile([P, MC, N], bf16, tag="et", name="ET")
                tl["G"] = work.tile([P, NC, D], bf16, tag="g", name="G")
                tl["Hs"] = work.tile([P, MC, D], bf16, tag="hs", name="Hs")
                tl["rs"] = work.tile([P, NC], f32, tag="rs", name="rs")
                tl["rr"] = work.tile([P, NC], f32, tag="rr", name="rr")
                return tl

            def load(tl):
                b = tl["b"]
                for c in range(NC):
                    nc.sync.dma_start(
                        out=tl["Cb"][:, c, :], in_=Cd[b, c * P : (c + 1) * P, :]
                    )
                for mm in range(MC):
                    nc.sync.dma_start(
                        out=tl["Qb"][:, mm, :], in_=Qd[b, mm * P : (mm + 1) * P, :]
                    )

            def gen_transposes(tl):
                """Yield after each PE transpose. QT first (sim needs all of
                it), then CT in two n-half groups so sim[0..3] unblocks
                after the first. Drains are one [P,512] instr per group."""
                Cb, Qb, CT, QT = tl["Cb"], tl["Qb"], tl["CT"], tl["QT"]
                for e in range(DC):
                    tp = ps_t.tile([P, M], bf16, tag="t", name="tpq")
                    for mm in range(MC):
                        nc.tensor.transpose(
                            tp[:, mm * P : (mm + 1) * P],
                            Qb[:, mm, e * P : (e + 1) * P],
                            ident_b,
                        )
                        yield
                    nc.vector.tensor_copy(out=QT[:, e, :], in_=tp)
                for cg in range(2):
                    for e in range(DC):
                        tp = ps_t.tile([P, 4 * P], bf16, tag="t", name="tpc")
                        for j in range(4):
                            c = cg * 4 + j
                            nc.tensor.transpose(
                                tp[:, j * P : (j + 1) * P],
                                Cb[:, c, e * P : (e + 1) * P],
                                ident_b,
                            )
                            yield
                        # C' = C*w4mlu + w4Q applied on the d-part drain
                        nc.vector.tensor_scalar(
                            out=CT[:, e, cg * 512 : (cg + 1) * 512],
                            in0=tp,
                            scalar1=wmlu_pp[:, e : e + 1],
                            scalar2=w4Q_pp[:, e : e + 1],
                            op0=ALU.mult,
                            op1=ALU.add,
                        )

            def emit_warm(junk_ps):
                # real matmul to keep the PE HAM clock from gating during
                # transpose-only stretches
                nc.tensor.matmul(
                    junk_ps[:, 0:P], lhsT=ident_b, rhs=ident_b, start=True, stop=True
                )

            def emit_te_h(tl, c, h_tiles):
                """E^T tiles for chunk c (4 transposes + 1 drain) and the H
                accumulation contribution of chunk c (4 matmuls)."""
                E, ET, G = tl["E"], tl["ET"], tl["G"]
                tp = ps_t.tile([P, MC, P], bf16, tag="t", name="tpe")
                for mm in range(MC):
                    nc.tensor.transpose(
                        tp[:, mm, :], E[:, c, mm * P : (mm + 1) * P], ident_b
                    )
                nc.vector.tensor_copy(out=ET[:, :, c * P : (c + 1) * P], in_=tp)
                for mm in range(MC):
                    nc.tensor.matmul(
                        h_tiles[mm],
                        lhsT=E[:, c, mm * P : (mm + 1) * P],
                        rhs=G[:, c, :],
                        start=(c == 0),
                        stop=(c == NC - 1),
                    )

            def emit_simloop(tl):
                """sim -> E (exp+rowsum fused) -> r -> G; E^T and H pipelined
                one chunk behind to hide the ACT/DVE latency."""
                CT, QT, E = tl["CT"], tl["QT"], tl["E"]
                rs, rr, G, Cb = tl["rs"], tl["rr"], tl["G"], tl["Cb"]
                h_tiles = [
                    ps_h.tile([P, D], f32, tag=f"h{mm}", name=f"h{mm}")
                    for mm in range(MC)
                ]
                for c in range(NC):
                    sim_ps = ps_sim.tile([P, M], f32, tag="sim", name="sim")
                    for e in range(DC):
                        nc.tensor.matmul(
                            sim_ps,
                            lhsT=CT[:, e, c * P : (c + 1) * P],
                            rhs=QT[:, e, :],
                            start=(e == 0),
                            stop=(e == DC - 1),
                        )
                    nc.scalar.activation(
                        out=E[:, c, :],
                        in_=sim_ps,
                        func=ACT.Exp,
                        accum_out=rs[:, c : c + 1],
                    )
                    nc.vector.reciprocal(out=rr[:, c : c + 1], in_=rs[:, c : c + 1])
                    nc.gpsimd.tensor_scalar_mul(
                        out=G[:, c, :], in0=Cb[:, c, :], scalar1=rr[:, c : c + 1]
                    )
                    if c > 0:
                        emit_te_h(tl, c - 1, h_tiles)
                emit_te_h(tl, NC - 1, h_tiles)
                for mm in range(MC):
                    nc.vector.tensor_copy(out=tl["Hs"][:, mm, :], in_=h_tiles[mm])

            def emit_ab(tl, interleave=None):
                """A = diag(r) E Q and Bv = diag(r) E H per n-chunk, then the
                elementwise outputs + DMA. Next batch's input transposes are
                interleaved 6-per-chunk to keep the PE warm."""
                b = tl["b"]
                ET, Qb, Hs, rr, Cb = tl["ET"], tl["Qb"], tl["Hs"], tl["rr"], tl["Cb"]
                for c in range(NC):
                    A_ps = ps_h.tile([P, D], f32, tag=f"h{c % 2}", name="Aps")
                    Bv_ps = ps_h.tile([P, D], f32, tag=f"h{2 + c % 2}", name="Bvps")
                    for mm in range(MC):
                        nc.tensor.matmul(
                            A_ps,
                            lhsT=ET[:, mm, c * P : (c + 1) * P],
                            rhs=Qb[:, mm, :],
                            start=(mm == 0),
                            stop=(mm == MC - 1),
                        )
                    for mm in range(MC):
                        nc.tensor.matmul(
                            Bv_ps,
                            lhsT=ET[:, mm, c * P : (c + 1) * P],
                            rhs=Hs[:, mm, :],
                            start=(mm == 0),
                            stop=(mm == MC - 1),
                        )
                    A_s = stage.tile([P, D], bf16, tag="a", name="A_s")
                    nc.scalar.activation(
                        out=A_s, in_=A_ps, func=ACT.Copy, scale=rr[:, c : c + 1]
                    )
                    Bv_s = stage.tile([P, D], bf16, tag="bv", name="Bv_s")
                    nc.scalar.activation(
                        out=Bv_s, in_=Bv_ps, func=ACT.Copy, scale=rr[:, c : c + 1]
                    )
                    CA_s = stage.tile([P, D], bf16, tag="ca", name="CA_s")
                    nc.vector.tensor_mul(out=CA_s, in0=Cb[:, c, :], in1=A_s)
                    CBv_s = stage.tile([P, D], bf16, tag="cbv", name="CBv_s")
                    nc.gpsimd.tensor_mul(out=CBv_s, in0=Cb[:, c, :], in1=Bv_s)
                    nc.sync.dma_start(out=Ad[b, c * P : (c + 1) * P, :], in_=A_s)
                    nc.sync.dma_start(out=CAd[b, c * P : (c + 1) * P, :], in_=CA_s)
                    nc.sync.dma_start(out=CBvd[b, c * P : (c + 1) * P, :], in_=CBv_s)
                    if interleave is not None:
                        for _ in range(6):
                            next(interleave, None)

            # ---- pipeline over the two batches ----
            tl0 = alloc(0)
            load(tl0)
            gen0 = gen_transposes(tl0)
            for i, _ in enumerate(gen0):
                if i % 3 == 2:
                    junk = ps_sim.tile([P, M], f32, tag="sim", name="junk")
                    emit_warm(junk)
            tl1 = alloc(1)
            load(tl1)
            emit_simloop(tl0)
            gen1 = gen_transposes(tl1)
            emit_ab(tl0, interleave=gen1)
            for _ in gen1:
                pass
            emit_simloop(tl1)
            emit_ab(tl1)

    nc.compile()
    return nc


def _reference_fallback(C, Q, Cmask, Qmask, w4C, w4Q, w4mlu, bias):
    """Numpy fallback for non-all-ones masks (not expected per spec)."""

    def softmax(x, axis):
        x = x - np.max(x, axis=axis, keepdims=True)
        e = np.exp(x)
        return e / np.sum(e, axis=axis, keepdims=True)

    sub0 = C @ w4C
    sub1 = np.swapaxes(Q @ w4Q, 1, 2)
    sub2 = np.einsum("bnd,bmd->bnm", C * w4mlu, Q)
    sim = sub0 + sub1 + sub2 + bias
    s1m = np.where(Qmask[:, None, :] == 0, -np.inf, sim)
    s2m = np.where(Cmask[:, :, None] == 0, -np.inf, sim)
    S1 = softmax(s1m, -1)
    S2 = softmax(s2m, -1)
    A = np.einsum("bnm,bmd->bnd", S1, Q)
    Bt = np.einsum("bnm,bkm->bnk", S1, S2)
    Bv = np.einsum("bnk,bkd->bnd", Bt, C)
    return np.concatenate([C, A, C * A, C * Bv], axis=2).astype(np.float32)


def kernel(C, Q, Cmask, Qmask, w4C, w4Q, w4mlu, bias):
    C = np.asarray(C, np.float32)
    Q = np.asarray(Q, np.float32)
    w4Q = np.asarray(w4Q, np.float32)
    w4mlu = np.asarray(w4mlu, np.float32)

    if not (np.all(np.asarray(Cmask) == 1) and np.all(np.asarray(Qmask) == 1)):
        return _reference_fallback(
            C,
            Q,
            np.asarray(Cmask),
            np.asarray(Qmask),
            np.asarray(w4C, np.float32),
            w4Q,
            w4mlu,
            np.asarray(bias, np.float32),
        )

    import os

    import ml_dtypes

    from concourse.bass_utils import run_bass_kernel_spmd

    if "nc" not in _cache:
        _cache["nc"] = _build()
    nc = _cache["nc"]

    bf = ml_dtypes.bfloat16
    Cb = C.astype(bf)
    Qb = Q.astype(bf)
    in_maps = []
    for i in range(NCORES):
        in_maps.append(
            {
                "C": np.ascontiguousarray(Cb[i * BPC : (i + 1) * BPC]),
                "Q": np.ascontiguousarray(Qb[i * BPC : (i + 1) * BPC]),
                "w4Q": np.ascontiguousarray(w4Q),
                "wmlu": np.ascontiguousarray(w4mlu),
            }
        )

    trace = bool(int(os.environ.get("BASS_KERNEL_TRACE", "0")))
    res = run_bass_kernel_spmd(
        nc, in_maps, core_ids=list(range(NCORES)), trace=trace
    )
    if trace:
        _cache["exec_time_ns"] = res.exec_time_ns
        _cache["trace"] = res.instructions_and_trace

    out = np.empty((B, N, 4 * D), np.float32)
    out[:, :, 0:D] = C
    for i, r in enumerate(res.results):
        sl = slice(i * BPC, (i + 1) * BPC)
        out[sl, :, D : 2 * D] = np.asarray(r["A"]).astype(np.float32)
        out[sl, :, 2 * D : 3 * D] = np.asarray(r["CA"]).astype(np.float32)
        out[sl, :, 3 * D : 4 * D] = np.asarray(r["CBv"]).astype(np.float32)
    return out


# revision 5
# speedup vs baseline: 2.0441x; 2.0441x over previous
"""CQAttention Trainium2 kernel: out = concat([C, A, C*A, C*Bv], -1).

Math (exact, given all-ones masks):
  - sub0 (per-row) and bias are constant along the softmax axis m -> cancel.
  - sub1[m] = sum_d Q[m,d] w4Q[d] folds into the score matmul exactly:
      sim[n,m] = sum_d (C[n,d]*w4mlu[d] + w4Q[d]) * Q[m,d] = sub2 + sub1
  - S1 == S2 == diag(r) E with E = exp(sim), r = 1/rowsum(E).
  - Reassociation halves the Bv cost:
      Bv = S1 S2^T C = diag(r) E (E^T (diag(r) C))
    i.e.  G = diag(r) C;  H = E^T G  (m-part);  Bv = diag(r) (E H).
  - A = diag(r) (E Q).

Implementation:
  - All operands bf16 (matmul rate equals f32r, transposes are cheaper,
    SBUF/DMA halved). exp is computed from an fp32 PSUM sim, so E is
    accurate to bf16 rounding. Tolerance is 2e-2; this lands ~1e-3.
  - exp + rowsum fused in one scalar-engine activation via accum_out.
  - E is produced n-part (lhsT for H); E^T (lhsT for A/Bv) via 32 PE
    transposes per batch, drained 4-at-a-time from one PSUM tile.
  - Host side: inputs cast to bf16, outputs (A, C*A, C*Bv) returned as
    bf16 and upcast on host; the C passthrough block is assembled on the
    host from the original f32 input (pure data movement, no compute).

Sharding: data-parallel over batch; core i handles batches [2i, 2i+1].
"""

import sys

if "/opt/trn_rl_repo" not in sys.path:
    sys.path.insert(0, "/opt/trn_rl_repo")

import numpy as np

B, N, M, D = 16, 1024, 512, 512
NCORES = 8
BPC = B // NCORES  # batches per core
P = 128
NC = N // P  # 8 n-chunks
MC = M // P  # 4 m-chunks
DC = D // P  # 4 d-chunks

_cache = {}


def _build():
    import concourse.bass as bass
    import concourse.tile as tile
    from concourse import bacc, mybir
    from concourse.masks import make_identity

    f32 = mybir.dt.float32
    bf16 = mybir.dt.bfloat16
    ACT = mybir.ActivationFunctionType
    ALU = mybir.AluOpType

    nc = bacc.Bacc("TRN2")
    Cd = nc.dram_tensor("C", [BPC, N, D], bf16, kind="ExternalInput")
    Qd = nc.dram_tensor("Q", [BPC, M, D], bf16, kind="ExternalInput")
    w4Qd = nc.dram_tensor("w4Q", [D, 1], f32, kind="ExternalInput")
    wmlud = nc.dram_tensor("wmlu", [1, 1, D], f32, kind="ExternalInput")
    Ad = nc.dram_tensor("A", [BPC, N, D], bf16, kind="ExternalOutput")
    CAd = nc.dram_tensor("CA", [BPC, N, D], bf16, kind="ExternalOutput")
    CBvd = nc.dram_tensor("CBv", [BPC, N, D], bf16, kind="ExternalOutput")

    with tile.TileContext(nc) as tc:
        with (
            tc.tile_pool(name="consts", bufs=1) as consts,
            tc.tile_pool(name="io", bufs=2) as io,
            tc.tile_pool(name="work", bufs=2) as work,
            tc.tile_pool(name="stage", bufs=3) as stage,
            tc.tile_pool(name="ps_sim", bufs=2, space="PSUM") as ps_sim,
            tc.tile_pool(name="ps_t", bufs=2, space="PSUM") as ps_t,
            tc.tile_pool(name="ps_h", bufs=1, space="PSUM") as ps_h,
        ):
            ident = consts.tile([P, P], f32, tag="ident")
            make_identity(nc, ident)
            ident_b = consts.tile([P, P], bf16, tag="identb")
            nc.vector.tensor_copy(out=ident_b, in_=ident)
            # per-partition weight tables, element [p, e] = w[e*128 + p]
            wmlu_pp = consts.tile([P, DC], f32, tag="wmlu")
            nc.gpsimd.dma_start(
                out=wmlu_pp, in_=bass.AP(tensor=wmlud, offset=0, ap=[[1, P], [P, DC]])
            )
            w4Q_pp = consts.tile([P, DC], f32, tag="w4q")
            nc.gpsimd.dma_start(
                out=w4Q_pp, in_=bass.AP(tensor=w4Qd, offset=0, ap=[[1, P], [P, DC]])
            )

            def alloc(b):
                tl = {"b": b}
                tl["Cb"] = io.tile([P, NC, D], bf16, tag="cb", name="Cb")
                tl["Qb"] = io.tile([P, MC, D], bf16, tag="qb", name="Qb")
                tl["CT"] = work.tile([P, DC, N], bf16, tag="ct", name="CT")
                tl["QT"] = work.tile([P, DC, M], bf16, tag="qt", name="QT")
                tl["E"] = work.tile([P, NC, M], bf16, tag="e", name="E")
                tl["ET"] = work.tile([P, MC, N], bf16, tag="et", name="ET")
                tl["G"] = work.tile([P, NC, D], bf16, tag="g", name="G")
                tl["Hs"] = work.tile([P, MC, D], bf16, tag="hs", name="Hs")
                tl["rs"] = work.tile([P, NC], f32, tag="rs", name="rs")
                tl["rr"] = work.tile([P, NC], f32, tag="rr", name="rr")
                return tl

            def load(tl):
                b = tl["b"]
                for c in range(NC):
                    nc.sync.dma_start(
                        out=tl["Cb"][:, c, :], in_=Cd[b, c * P : (c + 1) * P, :]
                    )
                for mm in range(MC):
                    nc.sync.dma_start(
                        out=tl["Qb"][:, mm, :], in_=Qd[b, mm * P : (mm + 1) * P, :]
                    )

            def gen_transposes(tl):
                """Yield after each PE transpose. QT first (sim needs all of
                it), then CT in two n-half groups so sim[0..3] unblocks
                after the first. Drains are one [P,512] instr per group."""
                Cb, Qb, CT, QT = tl["Cb"], tl["Qb"], tl["CT"], tl["QT"]
                for e in range(DC):
                    tp = ps_t.tile([P, M], bf16, tag="t", name="tpq")
                    for mm in range(MC):
                        nc.tensor.transpose(
                            tp[:, mm * P : (mm + 1) * P],
                            Qb[:, mm, e * P : (e + 1) * P],
                            ident_b,
                        )
                        yield
                    nc.vector.tensor_copy(out=QT[:, e, :], in_=tp)
                for cg in range(2):
                    for e in range(DC):
                        tp = ps_t.tile([P, 4 * P], bf16, tag="t", name="tpc")
                        for j in range(4):
                            c = cg * 4 + j
                            nc.tensor.transpose(
                                tp[:, j * P : (j + 1) * P],
                                Cb[:, c, e * P : (e + 1) * P],
                                ident_b,
                            )
                            yield
                        # C' = C*w4mlu + w4Q applied on the d-part drain
                        nc.vector.tensor_scalar(
                            out=CT[:, e, cg * 512 : (cg + 1) * 512],
                            in0=tp,
                            scalar1=wmlu_pp[:, e : e + 1],
                            scalar2=w4Q_pp[:, e : e + 1],
                            op0=ALU.mult,
                            op1=ALU.add,
                        )

            def emit_warm(junk_ps):
                # real matmul to keep the PE HAM clock from gating during
                # transpose-only stretches
                nc.tensor.matmul(
                    junk_ps[:, 0:P], lhsT=ident_b, rhs=ident_b, start=True, stop=True
                )

            def emit_te_h(tl, c, h_tiles):
                """E^T tiles for chunk c (4 transposes + 1 drain) and the H
                accumulation contribution of chunk c (4 matmuls)."""
                E, ET, G = tl["E"], tl["ET"], tl["G"]
                tp = ps_t.tile([P, MC, P], bf16, tag="t", name="tpe")
                for mm in range(MC):
                    nc.tensor.transpose(
                        tp[:, mm, :], E[:, c, mm * P : (mm + 1) * P], ident_b
                    )
                nc.vector.tensor_copy(out=ET[:, :, c * P : (c + 1) * P], in_=tp)
                for mm in range(MC):
                    nc.tensor.matmul(
                        h_tiles[mm],
                        lhsT=E[:, c, mm * P : (mm + 1) * P],
                        rhs=G[:, c, :],
                        start=(c == 0),
                        stop=(c == NC - 1),
                    )

            def emit_simloop(tl):
                """sim -> E (exp+rowsum fused) -> r -> G; E^T and H pipelined
                one chunk behind to hide the ACT/DVE latency."""
                CT, QT, E = tl["CT"], tl["QT"], tl["E"]
                rs, rr, G, Cb = tl["rs"], tl["rr"], tl["G"], tl["Cb"]
                h_tiles = [
                    ps_h.tile([P, D], f32, tag=f"h{mm}", name=f"h{mm}")
                    for mm in range(MC)
                ]
                for c in range(NC):
                    sim_ps = ps_sim.tile([P, M], f32, tag="sim", name="sim")
                    for e in range(DC):
                        nc.tensor.matmul(
                            sim_ps,
                            lhsT=CT[:, e, c * P : (c + 1) * P],
                            rhs=QT[:, e, :],
                            start=(e == 0),
                            stop=(e == DC - 1),
                        )
                    nc.scalar.activation(
                        out=E[:, c, :],
                        in_=sim_ps,
                        func=ACT.Exp,
                        accum_out=rs[:, c : c + 1],
                    )
                    nc.vector.reciprocal(out=rr[:, c : c + 1], in_=rs[:, c : c + 1])
                    nc.vector.tensor_scalar_mul(
                        out=G[:, c, :], in0=Cb[:, c, :], scalar1=rr[:, c : c + 1]
                    )
                    if c > 0:
                        emit_te_h(tl, c - 1, h_tiles)
                emit_te_h(tl, NC - 1, h_tiles)
                for mm in range(MC):
                    nc.scalar.copy(out=tl["Hs"][:, mm, :], in_=h_tiles[mm])

            def emit_ab(tl, interleave=None):
                """A = diag(r) E Q and Bv = diag(r) E H per n-chunk, then the
                elementwise outputs + DMA. Next batch's input transposes are
                interleaved 6-per-chunk to keep the PE warm."""
                b = tl["b"]
                ET, Qb, Hs, rr, Cb = tl["ET"], tl["Qb"], tl["Hs"], tl["rr"], tl["Cb"]
                for c in range(NC):
                    A_ps = ps_h.tile([P, D], f32, tag=f"h{c % 2}", name="Aps")
                    Bv_ps = ps_h.tile([P, D], f32, tag=f"h{2 + c % 2}", name="Bvps")
                    for mm in range(MC):
                        nc.tensor.matmul(
                            A_ps,
                            lhsT=ET[:, mm, c * P : (c + 1) * P],
                            rhs=Qb[:, mm, :],
                            start=(mm == 0),
                            stop=(mm == MC - 1),
                        )
                    for mm in range(MC):
                        nc.tensor.matmul(
                            Bv_ps,
                            lhsT=ET[:, mm, c * P : (c + 1) * P],
                            rhs=Hs[:, mm, :],
                            start=(mm == 0),
                            stop=(mm == MC - 1),
                        )
                    A_s = stage.tile([P, D], bf16, tag="a", name="A_s")
                    nc.scalar.activation(
                        out=A_s, in_=A_ps, func=ACT.Copy, scale=rr[:, c : c + 1]
                    )
                    Bv_s = stage.tile([P, D], bf16, tag="bv", name="Bv_s")
                    nc.scalar.activation(
                        out=Bv_s, in_=Bv_ps, func=ACT.Copy, scale=rr[:, c : c + 1]
                    )
                    CA_s = stage.tile([P, D], bf16, tag="ca", name="CA_s")
                    nc.vector.tensor_mul(out=CA_s, in0=Cb[:, c, :], in1=A_s)
                    CBv_s = stage.tile([P, D], bf16, tag="cbv", name="CBv_s")
                    nc.vector.tensor_mul(out=CBv_s, in0=Cb[:, c, :], in1=Bv_s)
                    nc.sync.dma_start(out=Ad[b, c * P : (c + 1) * P, :], in_=A_s)
                    nc.sync.dma_start(out=CAd[b, c * P : (c + 1) * P, :], in_=CA_s)
                    nc.sync.dma_start(out=CBvd[b, c * P : (c + 1) * P, :], in_=CBv_s)
                    if interleave is not None:
                        for _ in range(6):
                            next(interleave, None)

            # ---- pipeline over the two batches ----
            tl0 = alloc(0)
            load(tl0)
            gen0 = gen_transposes(tl0)
            for i, _ in enumerate(gen0):
                if i % 3 == 2:
                    junk = ps_sim.tile([P, M], f32, tag="sim", name="junk")
                    emit_warm(junk)
            tl1 = alloc(1)
            load(tl1)
            emit_simloop(tl0)
            gen1 = gen_transposes(tl1)
            emit_ab(tl0, interleave=gen1)
            for _ in gen1:
                pass
            emit_simloop(tl1)
            emit_ab(tl1)

    nc.compile()
    return nc


def _reference_fallback(C, Q, Cmask, Qmask, w4C, w4Q, w4mlu, bias):
    """Numpy fallback for non-all-ones masks (not expected per spec)."""

    def softmax(x, axis):
        x = x - np.max(x, axis=axis, keepdims=True)
        e = np.exp(x)
        return e / np.sum(e, axis=axis, keepdims=True)

    sub0 = C @ w4C
    sub1 = np.swapaxes(Q @ w4Q, 1, 2)
    sub2 = np.einsum("bnd,bmd->bnm", C * w4mlu, Q)
    sim = sub0 + sub1 + sub2 + bias
    s1m = np.where(Qmask[:, None, :] == 0, -np.inf, sim)
    s2m = np.where(Cmask[:, :, None] == 0, -np.inf, sim)
    S1 = softmax(s1m, -1)
    S2 = softmax(s2m, -1)
    A = np.einsum("bnm,bmd->bnd", S1, Q)
    Bt = np.einsum("bnm,bkm->bnk", S1, S2)
    Bv = np.einsum("bnk,bkd->bnd", Bt, C)
    return np.concatenate([C, A, C * A, C * Bv], axis=2).astype(np.float32)


def kernel(C, Q, Cmask, Qmask, w4C, w4Q, w4mlu, bias):
    C = np.asarray(C, np.float32)
    Q = np.asarray(Q, np.float32)
    w4Q = np.asarray(w4Q, np.float32)
    w4mlu = np.asarray(w4mlu, np.float32)

    if not (np.all(np.asarray(Cmask) == 1) and np.all(np.asarray(Qmask) == 1)):
        return _reference_fallback(
            C,
            Q,
            np.asarray(Cmask),
            np.asarray(Qmask),
            np.asarray(w4C, np.float32),
            w4Q,
            w4mlu,
            np.asarray(bias, np.float32),
        )

    import os

    import ml_dtypes

    from concourse.bass_utils import run_bass_kernel_spmd

    if "nc" not in _cache:
        _cache["nc"] = _build()
    nc = _cache["nc"]

    bf = ml_dtypes.bfloat16
    Cb = C.astype(bf)
    Qb = Q.astype(bf)
    in_maps = []
    for i in range(NCORES):
        in_maps.append(
            {
                "C": np.ascontiguousarray(Cb[i * BPC : (i + 1) * BPC]),
                "Q": np.ascontiguousarray(Qb[i * BPC : (i + 1) * BPC]),
                "w4Q": np.ascontiguousarray(w4Q),
                "wmlu": np.ascontiguousarray(w4mlu),
            }
        )

    trace = bool(int(os.environ.get("BASS_KERNEL_TRACE", "0")))
    res = run_bass_kernel_spmd(
        nc, in_maps, core_ids=list(range(NCORES)), trace=trace
    )
    if trace:
        _cache["exec_time_ns"] = res.exec_time_ns
        _cache["trace"] = res.instructions_and_trace

    out = np.empty((B, N, 4 * D), np.float32)
    out[:, :, 0:D] = C
    for i, r in enumerate(res.results):
        sl = slice(i * BPC, (i + 1) * BPC)
        out[sl, :, D : 2 * D] = np.asarray(r["A"]).astype(np.float32)
        out[sl, :, 2 * D : 3 * D] = np.asarray(r["CA"]).astype(np.float32)
        out[sl, :, 3 * D : 4 * D] = np.asarray(r["CBv"]).astype(np.float32)
    return out
